# revision 25
# baseline (speedup 1.0000x reference)
"""IterNorm (training-mode whitening, num_groups=1) Bass/Tile kernel for 8 trn2 cores.

Strategy (data-parallel over batch B, per sharding hint):
  - Each of the 8 cores gets 4 of the 32 batches: X_shard (4, 64, 8192) f32.
  - Batches are stacked in pairs onto 128 SBUF partitions (p0-63 = even batch
    channels, 64-127 = odd batch channels); full 128-partition HBM DMAs.
  - Stats pass, pipelined per tile: f32 load -> cast to an fp16 shadow split
    DVE/ACT with the per-channel row sums fused in via accum_out -> PE
    transposes in groups of 4 chunks into one PSUM bank -> one DVE copy per
    group -> accumulating 128x128 fp16 Gram matmul into f32 PSUM.  PE does
    ~163ns per 128-col chunk (transpose + matmul, weight loads overlapped),
    so the phase tracks the HBM load roofline.
  - The stacked (128,128) block + sums are folded locally to (64,65)
    (selector matmul) and combined across cores with a 16.6 KB AllGather +
    on-chip reduction.
  - Replicated epilogue: Sigma/trace with the DVE kept clear of bulk work,
    trace broadcast via one all-ones matmul; Newton-Schulz in fp16 with
    iteration 1 folded into P1 = 1.5I - 0.5 Sigma_N and 4 PE iterations of
    {P2|Q paired matmuls in one PSUM bank -> one DVE cast -> C matmuls}.
    W2 = blockdiag(wm, wm) built with two identity matmuls (PE can cross
    partitions; DVE cannot).
  - Apply pass: mu pre-subtracted from the fp16 shadows in place on ACT
    (hidden under the NS iterations), then per (128,512) chunk:
    y = W2 @ xc on PE -> PSUM->SBUF copy (DVE, ACT helps on the tail) ->
    per-chunk f32 store.

Notes vs. hardware: tensor_tensor_reduce crashes on hw (sim-only); GpSimd
ALU ops run ~10 G elem/s; the XBAR DMA-transpose ucode is descriptor-bound
(~1us per 128-col chunk per ring) -- all three are avoided.

Self-contained: hardcodes shapes and builds all constant inputs on the host.
"""

import sys

for _p in ("/opt/trn_rl_repo",):
    if _p not in sys.path:
        sys.path.insert(0, _p)

import numpy as np

import concourse.bass as bass  # noqa: F401
import concourse.tile as tile
from concourse import bacc, mybir
from concourse.bass_utils import run_bass_kernel_spmd

NCORES = 8
B, C, L = 32, 64, 8192
BPC = B // NCORES            # batches per core
M_TOT = B * L
T_NS = 5
F32 = mybir.dt.float32
F16 = mybir.dt.float16
XTILE_W = 2048
TILE_PLAN = [2048, 2048, 2048, 1536, 512]   # per pair; sums to L

_CACHE = {}


def _build_bass(ncores=NCORES):
    nc = bacc.Bacc("TRN2", target_bir_lowering=False, debug=False, num_devices=ncores)

    X = nc.dram_tensor("X", [BPC, C, L], F32, kind="ExternalInput")
    Y = nc.dram_tensor("Y", [BPC, C, L], F32, kind="ExternalOutput")
    # packed constants: one f32 and one f16 tensor (2 DMAs)
    CF32 = nc.dram_tensor("CF32", [128, 320], F32, kind="ExternalInput")
    CF16 = nc.dram_tensor("CF16", [128, 256], F16, kind="ExternalInput")

    Xv = X.ap().rearrange("(p i) c l -> p (i c) l", i=2)
    Yv = Y.ap().rearrange("(p i) c l -> p (i c) l", i=2)
    tile_geom = []
    for pair in range(2):
        off = 0
        for w in TILE_PLAN:
            tile_geom.append((pair, off, w))
            off += w
    n_chunks = 2 * L // 128
    ntiles = len(tile_geom)

    with tile.TileContext(nc) as tc:
        with (
            tc.tile_pool(name="consts", bufs=1) as consts,
            tc.tile_pool(name="xpool", bufs=4) as xpool,
            tc.tile_pool(name="xTpool", bufs=4) as xTpool,
            tc.tile_pool(name="xbpool", bufs=1) as xbpool,
            tc.tile_pool(name="ypool", bufs=6) as ypool,
            tc.tile_pool(name="small", bufs=2) as small,
            tc.tile_pool(name="psumS", bufs=1, space="PSUM") as psumS,
            tc.tile_pool(name="psumSm", bufs=3, space="PSUM") as psumSm,
            tc.tile_pool(name="psumA", bufs=4, space="PSUM") as psumA,
            tc.tile_pool(name="dramp", bufs=1, space="DRAM") as dramp,
        ):
            # ---- constants (packed: 2 DMAs, on sync ahead of the loads) ----
            cf16 = consts.tile([128, 256], F16)
            nc.sync.dma_start(cf16, CF16.ap())
            cf32 = consts.tile([128, 320], F32)
            nc.sync.dma_start(cf32, CF32.ap())
            esel = cf32[:, 0:64]            # (128,64) rows 64:128 = I64
            ident64 = cf32[0:64, 64:128]    # (64,64) I
            f2h = cf32[0:64, 128:256]       # (64,128) [I|I]
            ones64 = cf32[0:64, 256:320]    # (64,64) ones
            i15h = cf16[0:64, 0:64]         # (64,64) 1.5 I fp16
            ih16 = cf16[0:64, 64:128]       # (64,64) I fp16
            identb = cf16[:, 128:256]       # (128,128) I fp16
            W2 = consts.tile([128, 128], F16)
            nc.gpsimd.memset(W2, 0.0)

            # ---- phase 1: load + cast(+row sums) + PE transpose + Gram ----
            S_ps = psumS.tile([128, 128], F32)
            srow = consts.tile([128, 2 * ntiles], F32)

            xb_tiles = []
            gi = 0
            cpi = 0
            for t, (pair, off, w) in enumerate(tile_geom):
                nch = w // 128
                xt = xpool.tile([128, w], F32, tag="xt", name=f"xt{t}",
                                padded_shape=[128, XTILE_W])
                nc.sync.dma_start(xt, Xv[pair, :, off:off + w])
                # fp16 shadow; cast split 25/75 DVE/ACT, row sums fused
                xb = xbpool.tile([128, w], F16, tag=f"xb{t}", name=f"xb{t}", bufs=1)
                h = (nch // 4) * 128
                nc.vector.tensor_scalar(
                    xb[:, 0:h], xt[:, 0:h], 1.0, None, mybir.AluOpType.mult,
                    mybir.AluOpType.add, accum_out=srow[:, 2 * t:2 * t + 1],
                )
                nc.scalar.activation(
                    xb[:, h:w], xt[:, h:w],
                    func=mybir.ActivationFunctionType.Identity,
                    accum_out=srow[:, 2 * t + 1:2 * t + 2],
                )
                xb_tiles.append(xb)
                xTt = xTpool.tile([128, nch, 128], F16, tag="xT", name=f"xT{t}",
                                  padded_shape=[128, 16, 128])
                # PE transposes, groups of 8 chunks -> one PSUM bank -> 1 copy
                for g0 in range(0, nch, 8):
                    gn = min(8, nch - g0)
                    tp = psumA.tile([128, gn * 128], F16, tag="ap",
                                    name=f"tp{t}_{g0}", padded_shape=[128, 1024])
                    for k in range(gn):
                        nc.tensor.transpose(
                            tp[:, k * 128:(k + 1) * 128],
                            xb[:, (g0 + k) * 128:(g0 + k + 1) * 128], identb)
                    cp = tp.rearrange("p (a b) -> p a b", a=gn)
                    nc.vector.tensor_copy(xTt[:, g0:g0 + gn, :], cp)
                    cpi += 1
                for k in range(nch):
                    nc.tensor.matmul(
                        S_ps,
                        xTt[:, k, :],
                        xTt[:, k, :],
                        start=(gi == 0),
                        stop=(gi == n_chunks - 1),
                        skip_group_check=True,
                    )
                    gi += 1

            # ---- local fold to (64,65) + AllGather + on-chip reduce ----
            S_sb = small.tile([128, 129], F32, tag="ssb")
            nc.vector.tensor_copy(S_sb[:, 0:128], S_ps)
            nc.vector.tensor_reduce(
                S_sb[:, 128:129], srow, axis=mybir.AxisListType.X,
                op=mybir.AluOpType.add,
            )
            ef_ps = psumSm.tile([64, 129], F32, tag="sm", name="ef_ps")
            nc.tensor.matmul(ef_ps, esel, S_sb, start=True, stop=True)
            pack = small.tile([64, 65], F32, tag="pack")
            nc.vector.tensor_add(pack[:, 0:64], S_sb[0:64, 0:64], ef_ps[:, 64:128])
            nc.vector.tensor_add(pack[:, 64:65], S_sb[0:64, 128:129],
                                 ef_ps[:, 128:129])
            agin = dramp.tile([64, 65], F32, tag="agin")
            agred = dramp.tile([64, 65], F32, tag="agred")
            nc.sync.dma_start(agin, pack)
            nc.gpsimd.collective_compute(
                "AllReduce",
                mybir.AluOpType.add,
                replica_groups=[list(range(ncores))],
                ins=[agin.opt()],
                outs=[agred.opt()],
            )
            tot = small.tile([64, 65], F32, tag="tot")
            nc.sync.dma_start(tot, agred)

            # ---- epilogue: mu, Sigma, trace, Newton-Schulz (replicated) ----
            mu = small.tile([64, 1], F32, tag="mu")
            nc.vector.tensor_scalar_mul(mu, tot[:, 64:65], 1.0 / M_TOT)
            mur_ps = psumSm.tile([1, 64], F32, tag="sm", name="mur_ps")
            nc.tensor.matmul(mur_ps, mu, ident64, start=True, stop=True)
            mu2b_ps = psumSm.tile([128, 1], F32, tag="sm", name="mu2b_ps")
            nc.tensor.matmul(mu2b_ps, f2h, mu, start=True, stop=True)
            mu_row = small.tile([1, 64], F32, tag="murow")
            nc.vector.tensor_copy(mu_row, mur_ps)
            mu_row_negM = small.tile([1, 64], F32, tag="murown")
            nc.vector.tensor_scalar_mul(mu_row_negM, mur_ps, -float(M_TOT))
            mu2h = consts.tile([128, 1], F16)
            nc.vector.tensor_copy(mu2h, mu2b_ps)
            outer_ps = psumSm.tile([64, 64], F32, tag="sm", name="outer_ps")
            nc.tensor.matmul(outer_ps, mu_row_negM, mu_row, start=True, stop=True)
            Sigma = small.tile([64, 64], F32, tag="sigma")
            diagm = small.tile([64, 64], F32, tag="diagm")
            dred = small.tile([64, 1], F32, tag="dred")
            nc.vector.tensor_add(diagm, tot[:, 0:64], outer_ps)
            nc.vector.tensor_scalar_mul(Sigma, diagm, 1.0 / M_TOT)
            nc.vector.tensor_mul(diagm, Sigma, ident64)
            nc.vector.tensor_reduce(
                dred, diagm, axis=mybir.AxisListType.X, op=mybir.AluOpType.add
            )
            tr64_ps = psumSm.tile([64, 1], F32, tag="sm", name="tr64_ps")
            nc.tensor.matmul(tr64_ps, ones64, dred, start=True, stop=True)
            trm2 = small.tile([64, 1], F32, tag="trm2")
            nc.vector.tensor_scalar_mul(trm2, tr64_ps, -2.0)
            rtr_nh = small.tile([64, 1], F32, tag="rtrnh")   # = -0.5/tr
            nc.vector.reciprocal(rtr_nh, trm2)
            srtr = small.tile([64, 1], F32, tag="srtr")      # = sqrt(1/tr)
            nc.scalar.activation(srtr, rtr_nh,
                                 func=mybir.ActivationFunctionType.Sqrt,
                                 scale=-2.0)
            Sh = small.tile([64, 64], F16, tag="sh")         # = -0.5 Sigma_N
            nc.vector.tensor_scalar_mul(Sh, Sigma, rtr_nh)
            P = small.tile([64, 64], F16, tag="P", name="P1")
            nc.vector.tensor_add(P, i15h, Sh)                # NS iteration 1

            # ---- Newton-Schulz iterations 2..5 (fp16, paired PSUM mms) ----
            for it in range(T_NS - 1):
                last = it == T_NS - 2
                psAB = psumSm.tile([64, 128], F32, tag="sm", name=f"psAB{it}")
                nc.tensor.matmul(psAB[:, 0:64], P, P, start=True, stop=True,
                                 skip_group_check=True)
                nc.tensor.matmul(psAB[:, 64:128], P, Sh, start=True, stop=True,
                                 skip_group_check=True)
                PQ = small.tile([64, 128], F16, tag="PQ", name=f"PQ{it}")
                nc.vector.tensor_copy(PQ, psAB)
                psC = psumSm.tile([64, 64], F32, tag="sm", name=f"psC{it}")
                nc.tensor.matmul(psC, PQ[:, 0:64], PQ[:, 64:128],
                                 start=True, stop=False, skip_group_check=True)
                nc.tensor.matmul(psC, i15h, P, start=False, stop=True,
                                 skip_group_check=True)
                if last:
                    wmh = small.tile([64, 64], F16, tag="wmh")
                    nc.vector.tensor_scalar_mul(wmh, psC, srtr)
                else:
                    P = small.tile([64, 64], F16, tag="P", name=f"P{it + 2}")
                    nc.vector.tensor_copy(P, psC)

            # W2 = blockdiag(wm, wm): two identity matmuls (PE crosses
            # partitions; DVE cannot)
            W2ps = psumSm.tile([128, 64], F32, tag="sm", name="W2ps")
            nc.tensor.matmul(W2ps[0:64, :], wmh, ih16, start=True, stop=True,
                             skip_group_check=True)
            nc.tensor.matmul(W2ps[64:128, :], wmh, ih16, start=True, stop=True,
                             skip_group_check=True)
            nc.vector.tensor_copy(W2[0:64, 0:64], W2ps[0:64, :])
            nc.vector.tensor_copy(W2[64:128, 64:128], W2ps[64:128, :])
            # bias_col = -(W2 @ mu2): folded into the apply-phase copies
            bias_ps = psumSm.tile([128, 1], F32, tag="sm", name="bias_ps")
            nc.tensor.matmul(bias_ps, W2, mu2h, start=True, stop=True)
            bias_col = consts.tile([128, 1], F32)
            nc.vector.tensor_scalar_mul(bias_col, bias_ps, -1.0)

            # ---- phase 3: apply y = W2 @ xb - W2 @ mu per (128,512) chunk ----
            ci = 0
            for t, (pair, off, w) in enumerate(tile_geom):
                for cidx in range(w // 512):
                    ap_ps = psumA.tile([128, 512], F32, tag="ap",
                                       name=f"ap{t}_{cidx}",
                                       padded_shape=[128, 512])
                    nc.tensor.matmul(
                        ap_ps,
                        W2,
                        xb_tiles[t][:, cidx * 512:(cidx + 1) * 512],
                        start=True,
                        stop=True,
                    )
                    yt = ypool.tile([128, 512], F32, tag="yt",
                                    name=f"yt{t}_{cidx}")
                    if ci % 2 == 0:
                        nc.vector.tensor_scalar_add(yt, ap_ps, bias_col)
                    else:
                        nc.scalar.activation(
                            yt, ap_ps,
                            func=mybir.ActivationFunctionType.Identity,
                            bias=bias_col, scale=1.0,
                        )
                    co = off + cidx * 512
                    nc.sync.dma_start(Yv[pair, :, co:co + 512], yt)
                    ci += 1

    nc.finalize()
    return nc


def _host_consts():
    i64 = np.eye(64, dtype=np.float32)
    cf32 = np.zeros((128, 320), dtype=np.float32)
    cf32[64:128, 0:64] = i64                          # esel
    cf32[0:64, 64:128] = i64                          # ident64
    cf32[0:64, 128:192] = i64                         # f2h left
    cf32[0:64, 192:256] = i64                         # f2h right
    cf32[0:64, 256:320] = 1.0                         # ones64
    cf16 = np.zeros((128, 256), dtype=np.float16)
    cf16[0:64, 0:64] = (1.5 * i64).astype(np.float16)  # i15h
    cf16[0:64, 64:128] = i64.astype(np.float16)        # ih16
    cf16[:, 128:256] = np.eye(128, dtype=np.float16)   # identb
    return {"CF32": cf32, "CF16": cf16}


NCORES_RUN = NCORES


def _get_nc():
    key = f"nc{NCORES_RUN}"
    if key not in _CACHE:
        _CACHE[key] = _build_bass(NCORES_RUN)
    return _CACHE[key]


def run(X, **spmd_kwargs):
    """Run the SPMD kernel; returns (Y_full, BassKernelResults)."""
    X = np.ascontiguousarray(np.asarray(X), dtype=np.float32)
    assert X.shape == (B, C, L), X.shape
    nc = _get_nc()
    consts = _host_consts()
    n = NCORES_RUN
    in_maps = [
        {"X": X[c * BPC:(c + 1) * BPC], **consts} for c in range(n)
    ]
    res = run_bass_kernel_spmd(nc, in_maps, core_ids=list(range(n)), **spmd_kwargs)
    Y = np.concatenate([res.results[c]["Y"] for c in range(n)], axis=0)
    return Y, res


def kernel(X):
    Y, _ = run(X)
    return Y


# revision 26
# speedup vs baseline: 1.0351x; 1.0351x over previous
"""IterNorm (training-mode whitening, num_groups=1) Bass/Tile kernel for 8 trn2 cores.

Strategy (data-parallel over batch B, per sharding hint):
  - Each of the 8 cores gets 4 of the 32 batches: X_shard (4, 64, 8192) f32.
  - Batches are stacked in pairs onto 128 SBUF partitions (p0-63 = even batch
    channels, 64-127 = odd batch channels); full 128-partition HBM DMAs.
  - Stats pass, pipelined per tile: f32 load -> cast to an fp16 shadow split
    DVE/ACT with the per-channel row sums fused in via accum_out -> PE
    transposes in groups of 4 chunks into one PSUM bank -> one DVE copy per
    group -> accumulating 128x128 fp16 Gram matmul into f32 PSUM.  PE does
    ~163ns per 128-col chunk (transpose + matmul, weight loads overlapped),
    so the phase tracks the HBM load roofline.
  - The stacked (128,128) block + sums are folded locally to (64,65)
    (selector matmul) and combined across cores with a 16.6 KB AllGather +
    on-chip reduction.
  - Replicated epilogue: Sigma/trace with the DVE kept clear of bulk work,
    trace broadcast via one all-ones matmul; Newton-Schulz in fp16 with
    iteration 1 folded into P1 = 1.5I - 0.5 Sigma_N and 4 PE iterations of
    {P2|Q paired matmuls in one PSUM bank -> one DVE cast -> C matmuls}.
    W2 = blockdiag(wm, wm) built with two identity matmuls (PE can cross
    partitions; DVE cannot).
  - Apply pass: mu pre-subtracted from the fp16 shadows in place on ACT
    (hidden under the NS iterations), then per (128,512) chunk:
    y = W2 @ xc on PE -> PSUM->SBUF copy (DVE, ACT helps on the tail) ->
    per-chunk f32 store.

Notes vs. hardware: tensor_tensor_reduce crashes on hw (sim-only); GpSimd
ALU ops run ~10 G elem/s; the XBAR DMA-transpose ucode is descriptor-bound
(~1us per 128-col chunk per ring) -- all three are avoided.

Self-contained: hardcodes shapes and builds all constant inputs on the host.
"""

import sys

for _p in ("/opt/trn_rl_repo",):
    if _p not in sys.path:
        sys.path.insert(0, _p)

import numpy as np

import concourse.bass as bass  # noqa: F401
import concourse.tile as tile
from concourse import bacc, mybir
from concourse.bass_utils import run_bass_kernel_spmd

NCORES = 8
B, C, L = 32, 64, 8192
BPC = B // NCORES            # batches per core
M_TOT = B * L
T_NS = 5
F32 = mybir.dt.float32
F16 = mybir.dt.float16
XTILE_W = 2048
TILE_PLAN = [2048, 2048, 2048, 1536, 512]   # per pair; sums to L

_CACHE = {}


def _build_bass(ncores=NCORES):
    nc = bacc.Bacc("TRN2", target_bir_lowering=False, debug=False, num_devices=ncores)

    X = nc.dram_tensor("X", [BPC, C, L], F32, kind="ExternalInput")
    Y = nc.dram_tensor("Y", [BPC, C, L], F32, kind="ExternalOutput")
    # packed constants: one f32 and one f16 tensor (2 DMAs)
    CF32 = nc.dram_tensor("CF32", [128, 320], F32, kind="ExternalInput")
    CF16 = nc.dram_tensor("CF16", [128, 256], F16, kind="ExternalInput")

    Xv = X.ap().rearrange("(p i) c l -> p (i c) l", i=2)
    Yv = Y.ap().rearrange("(p i) c l -> p (i c) l", i=2)
    tile_geom = []
    for pair in range(2):
        off = 0
        for w in TILE_PLAN:
            tile_geom.append((pair, off, w))
            off += w
    n_chunks = 2 * L // 128
    ntiles = len(tile_geom)

    with tile.TileContext(nc) as tc:
        with (
            tc.tile_pool(name="consts", bufs=1) as consts,
            tc.tile_pool(name="xpool", bufs=4) as xpool,
            tc.tile_pool(name="xTpool", bufs=4) as xTpool,
            tc.tile_pool(name="xbpool", bufs=1) as xbpool,
            tc.tile_pool(name="ypool", bufs=6) as ypool,
            tc.tile_pool(name="small", bufs=2) as small,
            tc.tile_pool(name="psumS", bufs=1, space="PSUM") as psumS,
            tc.tile_pool(name="psumSm", bufs=3, space="PSUM") as psumSm,
            tc.tile_pool(name="psumA", bufs=4, space="PSUM") as psumA,
            tc.tile_pool(name="dramp", bufs=1, space="DRAM") as dramp,
        ):
            # ---- constants (packed: 2 DMAs, on sync ahead of the loads) ----
            cf16 = consts.tile([128, 256], F16)
            nc.sync.dma_start(cf16, CF16.ap())
            cf32 = consts.tile([128, 320], F32)
            nc.sync.dma_start(cf32, CF32.ap())
            esel = cf32[:, 0:64]            # (128,64) rows 64:128 = I64
            ident64 = cf32[0:64, 64:128]    # (64,64) I
            f2h = cf32[0:64, 128:256]       # (64,128) [I|I]
            ones64 = cf32[0:64, 256:320]    # (64,64) ones
            i15h = cf16[0:64, 0:64]         # (64,64) 1.5 I fp16
            ih16 = cf16[0:64, 64:128]       # (64,64) I fp16
            identb = cf16[:, 128:256]       # (128,128) I fp16
            W2 = consts.tile([128, 128], F16)
            nc.gpsimd.memset(W2, 0.0)

            # ---- phase 1: load + cast(+row sums) + PE transpose + Gram ----
            S_ps = psumS.tile([128, 128], F32)
            srow = consts.tile([128, 2 * ntiles], F32)

            xb_tiles = []
            gi = 0
            cpi = 0
            for t, (pair, off, w) in enumerate(tile_geom):
                nch = w // 128
                xt = xpool.tile([128, w], F32, tag="xt", name=f"xt{t}",
                                padded_shape=[128, XTILE_W])
                nc.sync.dma_start(xt, Xv[pair, :, off:off + w])
                # fp16 shadow; cast split 25/75 DVE/ACT, row sums fused
                xb = xbpool.tile([128, w], F16, tag=f"xb{t}", name=f"xb{t}", bufs=1)
                h = (nch // 4) * 128
                nc.vector.tensor_scalar(
                    xb[:, 0:h], xt[:, 0:h], 1.0, None, mybir.AluOpType.mult,
                    mybir.AluOpType.add, accum_out=srow[:, 2 * t:2 * t + 1],
                )
                nc.scalar.activation(
                    xb[:, h:w], xt[:, h:w],
                    func=mybir.ActivationFunctionType.Identity,
                    accum_out=srow[:, 2 * t + 1:2 * t + 2],
                )
                xb_tiles.append(xb)
                xTt = xTpool.tile([128, nch, 128], F16, tag="xT", name=f"xT{t}",
                                  padded_shape=[128, 16, 128])
                # PE transposes, groups of 8 chunks -> one PSUM bank -> 1 copy
                for g0 in range(0, nch, 8):
                    gn = min(8, nch - g0)
                    tp = psumA.tile([128, gn * 128], F16, tag="ap",
                                    name=f"tp{t}_{g0}", padded_shape=[128, 1024])
                    for k in range(gn):
                        nc.tensor.transpose(
                            tp[:, k * 128:(k + 1) * 128],
                            xb[:, (g0 + k) * 128:(g0 + k + 1) * 128], identb)
                    cp = tp.rearrange("p (a b) -> p a b", a=gn)
                    nc.vector.tensor_copy(xTt[:, g0:g0 + gn, :], cp)
                    cpi += 1
                for k in range(nch):
                    nc.tensor.matmul(
                        S_ps,
                        xTt[:, k, :],
                        xTt[:, k, :],
                        start=(gi == 0),
                        stop=(gi == n_chunks - 1),
                        skip_group_check=True,
                    )
                    gi += 1

            # ---- local fold to (64,65) + AllGather + on-chip reduce ----
            S_sb = small.tile([128, 129], F32, tag="ssb")
            nc.vector.tensor_copy(S_sb[:, 0:128], S_ps)
            nc.vector.tensor_reduce(
                S_sb[:, 128:129], srow, axis=mybir.AxisListType.X,
                op=mybir.AluOpType.add,
            )
            ef_ps = psumSm.tile([64, 129], F32, tag="sm", name="ef_ps")
            nc.tensor.matmul(ef_ps, esel, S_sb, start=True, stop=True)
            pack = small.tile([64, 65], F32, tag="pack")
            nc.vector.tensor_add(pack[:, 0:64], S_sb[0:64, 0:64], ef_ps[:, 64:128])
            nc.vector.tensor_add(pack[:, 64:65], S_sb[0:64, 128:129],
                                 ef_ps[:, 128:129])
            agin = dramp.tile([64, 65], F32, tag="agin")
            agout = dramp.tile([64 * ncores, 65], F32, tag="agout")
            nc.sync.dma_start(agin, pack)
            nc.gpsimd.collective_compute(
                "AllGather",
                mybir.AluOpType.bypass,
                replica_groups=[list(range(ncores))],
                ins=[agin.opt()],
                outs=[agout.opt()],
            )
            gath = small.tile([64, ncores, 65], F32, tag="gath")
            nc.sync.dma_start(
                gath, agout.rearrange("(k c) n -> c k n", k=ncores)
            )
            tot = small.tile([64, 65], F32, tag="tot")
            gv_sum = bass.AP(tensor=gath.tensor, offset=gath.offset + 64,
                             ap=[gath.ap[0], [1, 1], [65, ncores]])
            nc.vector.tensor_reduce(
                tot[:, 64:65], gv_sum, axis=mybir.AxisListType.X,
                op=mybir.AluOpType.add
            )
            gv_mat = bass.AP(tensor=gath.tensor, offset=gath.offset,
                             ap=[gath.ap[0], [1, 64], [65, ncores]])
            nc.vector.tensor_reduce(
                tot[:, 0:64], gv_mat, axis=mybir.AxisListType.X,
                op=mybir.AluOpType.add
            )

            # ---- epilogue: mu, Sigma, trace, Newton-Schulz (replicated) ----
            mu = small.tile([64, 1], F32, tag="mu")
            nc.vector.tensor_scalar_mul(mu, tot[:, 64:65], 1.0 / M_TOT)
            mur_ps = psumSm.tile([1, 64], F32, tag="sm", name="mur_ps")
            nc.tensor.matmul(mur_ps, mu, ident64, start=True, stop=True)
            mu2b_ps = psumSm.tile([128, 1], F32, tag="sm", name="mu2b_ps")
            nc.tensor.matmul(mu2b_ps, f2h, mu, start=True, stop=True)
            mu_row = small.tile([1, 64], F32, tag="murow")
            nc.vector.tensor_copy(mu_row, mur_ps)
            mu_row_negM = small.tile([1, 64], F32, tag="murown")
            nc.vector.tensor_scalar_mul(mu_row_negM, mur_ps, -float(M_TOT))
            mu2h = consts.tile([128, 1], F16)
            nc.vector.tensor_copy(mu2h, mu2b_ps)
            outer_ps = psumSm.tile([64, 64], F32, tag="sm", name="outer_ps")
            nc.tensor.matmul(outer_ps, mu_row_negM, mu_row, start=True, stop=True)
            Sigma = small.tile([64, 64], F32, tag="sigma")
            diagm = small.tile([64, 64], F32, tag="diagm")
            dred = small.tile([64, 1], F32, tag="dred")
            nc.vector.tensor_add(diagm, tot[:, 0:64], outer_ps)
            nc.vector.tensor_scalar_mul(Sigma, diagm, 1.0 / M_TOT)
            nc.vector.tensor_mul(diagm, Sigma, ident64)
            nc.vector.tensor_reduce(
                dred, diagm, axis=mybir.AxisListType.X, op=mybir.AluOpType.add
            )
            tr64_ps = psumSm.tile([64, 1], F32, tag="sm", name="tr64_ps")
            nc.tensor.matmul(tr64_ps, ones64, dred, start=True, stop=True)
            trm2 = small.tile([64, 1], F32, tag="trm2")
            nc.vector.tensor_scalar_mul(trm2, tr64_ps, -2.0)
            rtr_nh = small.tile([64, 1], F32, tag="rtrnh")   # = -0.5/tr
            nc.vector.reciprocal(rtr_nh, trm2)
            srtr = small.tile([64, 1], F32, tag="srtr")      # = sqrt(1/tr)
            nc.scalar.activation(srtr, rtr_nh,
                                 func=mybir.ActivationFunctionType.Sqrt,
                                 scale=-2.0)
            Sh = small.tile([64, 64], F16, tag="sh")         # = -0.5 Sigma_N
            nc.vector.tensor_scalar_mul(Sh, Sigma, rtr_nh)
            P = small.tile([64, 64], F16, tag="P", name="P1")
            nc.vector.tensor_add(P, i15h, Sh)                # NS iteration 1

            # ---- Newton-Schulz iterations 2..5 (fp16, paired PSUM mms) ----
            for it in range(T_NS - 1):
                last = it == T_NS - 2
                psAB = psumSm.tile([64, 128], F32, tag="sm", name=f"psAB{it}")
                nc.tensor.matmul(psAB[:, 0:64], P, P, start=True, stop=True,
                                 skip_group_check=True)
                nc.tensor.matmul(psAB[:, 64:128], P, Sh, start=True, stop=True,
                                 skip_group_check=True)
                PQ = small.tile([64, 128], F16, tag="PQ", name=f"PQ{it}")
                nc.vector.tensor_copy(PQ, psAB)
                psC = psumSm.tile([64, 64], F32, tag="sm", name=f"psC{it}")
                nc.tensor.matmul(psC, PQ[:, 0:64], PQ[:, 64:128],
                                 start=True, stop=False, skip_group_check=True)
                nc.tensor.matmul(psC, i15h, P, start=False, stop=True,
                                 skip_group_check=True)
                if last:
                    wmh = small.tile([64, 64], F16, tag="wmh")
                    nc.vector.tensor_scalar_mul(wmh, psC, srtr)
                else:
                    P = small.tile([64, 64], F16, tag="P", name=f"P{it + 2}")
                    nc.vector.tensor_copy(P, psC)

            # W2 = blockdiag(wm, wm): two identity matmuls (PE crosses
            # partitions; DVE cannot)
            W2ps = psumSm.tile([128, 64], F32, tag="sm", name="W2ps")
            nc.tensor.matmul(W2ps[0:64, :], wmh, ih16, start=True, stop=True,
                             skip_group_check=True)
            nc.tensor.matmul(W2ps[64:128, :], wmh, ih16, start=True, stop=True,
                             skip_group_check=True)
            nc.vector.tensor_copy(W2[0:64, 0:64], W2ps[0:64, :])
            nc.vector.tensor_copy(W2[64:128, 64:128], W2ps[64:128, :])
            # bias_col = -(W2 @ mu2): folded into the apply-phase copies
            bias_ps = psumSm.tile([128, 1], F32, tag="sm", name="bias_ps")
            nc.tensor.matmul(bias_ps, W2, mu2h, start=True, stop=True)
            bias_col = consts.tile([128, 1], F32)
            nc.vector.tensor_scalar_mul(bias_col, bias_ps, -1.0)

            # ---- phase 3: apply y = W2 @ xb - W2 @ mu per (128,512) chunk ----
            ci = 0
            for t, (pair, off, w) in enumerate(tile_geom):
                for cidx in range(w // 512):
                    ap_ps = psumA.tile([128, 512], F32, tag="ap",
                                       name=f"ap{t}_{cidx}",
                                       padded_shape=[128, 512])
                    nc.tensor.matmul(
                        ap_ps,
                        W2,
                        xb_tiles[t][:, cidx * 512:(cidx + 1) * 512],
                        start=True,
                        stop=True,
                    )
                    yt = ypool.tile([128, 512], F32, tag="yt",
                                    name=f"yt{t}_{cidx}")
                    if ci % 2 == 0:
                        nc.vector.tensor_scalar_add(yt, ap_ps, bias_col)
                    else:
                        nc.scalar.activation(
                            yt, ap_ps,
                            func=mybir.ActivationFunctionType.Identity,
                            bias=bias_col, scale=1.0,
                        )
                    co = off + cidx * 512
                    nc.sync.dma_start(Yv[pair, :, co:co + 512], yt)
                    ci += 1

    nc.finalize()
    return nc


def _host_consts():
    i64 = np.eye(64, dtype=np.float32)
    cf32 = np.zeros((128, 320), dtype=np.float32)
    cf32[64:128, 0:64] = i64                          # esel
    cf32[0:64, 64:128] = i64                          # ident64
    cf32[0:64, 128:192] = i64                         # f2h left
    cf32[0:64, 192:256] = i64                         # f2h right
    cf32[0:64, 256:320] = 1.0                         # ones64
    cf16 = np.zeros((128, 256), dtype=np.float16)
    cf16[0:64, 0:64] = (1.5 * i64).astype(np.float16)  # i15h
    cf16[0:64, 64:128] = i64.astype(np.float16)        # ih16
    cf16[:, 128:256] = np.eye(128, dtype=np.float16)   # identb
    return {"CF32": cf32, "CF16": cf16}


NCORES_RUN = NCORES


def _get_nc():
    key = f"nc{NCORES_RUN}"
    if key not in _CACHE:
        _CACHE[key] = _build_bass(NCORES_RUN)
    return _CACHE[key]


def run(X, **spmd_kwargs):
    """Run the SPMD kernel; returns (Y_full, BassKernelResults)."""
    X = np.ascontiguousarray(np.asarray(X), dtype=np.float32)
    assert X.shape == (B, C, L), X.shape
    nc = _get_nc()
    consts = _host_consts()
    n = NCORES_RUN
    in_maps = [
        {"X": X[c * BPC:(c + 1) * BPC], **consts} for c in range(n)
    ]
    res = run_bass_kernel_spmd(nc, in_maps, core_ids=list(range(n)), **spmd_kwargs)
    Y = np.concatenate([res.results[c]["Y"] for c in range(n)], axis=0)
    return Y, res


def kernel(X):
    Y, _ = run(X)
    return Y


# revision 27
# speedup vs baseline: 1.0627x; 1.0266x over previous
"""IterNorm (training-mode whitening, num_groups=1) Bass/Tile kernel for 8 trn2 cores.

Strategy (data-parallel over batch B, per sharding hint):
  - Each of the 8 cores gets 4 of the 32 batches: X_shard (4, 64, 8192) f32.
  - Batches are stacked in pairs onto 128 SBUF partitions (p0-63 = even batch
    channels, 64-127 = odd batch channels); full 128-partition HBM DMAs.
  - Stats pass, pipelined per tile: f32 load -> cast to an fp16 shadow split
    25/75 DVE/ACT with the per-channel row sums fused in via accum_out -> PE
    transposes in groups of 8 chunks into one PSUM bank -> one DVE copy per
    group -> accumulating 128x128 fp16 Gram matmul into f32 PSUM.  PE does
    ~163ns per 128-col chunk (transpose + matmul, weight loads overlapped),
    so the phase tracks the HBM load roofline.
  - The stacked (128,128) block + sums are folded locally to (64,65)
    (selector matmul) and combined across cores with a 16.6 KB AllGather +
    on-chip reduction.
  - Replicated epilogue: Sigma/trace with the DVE kept clear of bulk work,
    trace broadcast via one all-ones matmul; Newton-Schulz in fp16 with
    iteration 1 folded into P1 = 1.5I - 0.5 Sigma_N and 4 PE iterations of
    {P2|Q paired matmuls in one PSUM bank -> one DVE cast -> C matmuls}.
    W2 = blockdiag(wm, wm) built with two identity matmuls (PE can cross
    partitions; DVE cannot).
  - Apply pass, per (128,512) chunk: y = W2 @ xb on PE -> PSUM->SBUF copy
    with the -(W2 @ mu) bias folded in (alternating DVE tensor_scalar_add /
    ACT Identity+bias) -> per-chunk f32 store.  AllReduce was measured 2.7x
    slower than AllGather+reduce on the CC engine and is not used.

Notes vs. hardware: tensor_tensor_reduce crashes on hw (sim-only); GpSimd
ALU ops run ~10 G elem/s; the XBAR DMA-transpose ucode is descriptor-bound
(~1us per 128-col chunk per ring) -- all three are avoided.

Self-contained: hardcodes shapes and builds all constant inputs on the host.
"""

import sys

for _p in ("/opt/trn_rl_repo",):
    if _p not in sys.path:
        sys.path.insert(0, _p)

import numpy as np

import concourse.bass as bass  # noqa: F401
import concourse.tile as tile
from concourse import bacc, mybir
from concourse.bass_utils import run_bass_kernel_spmd

NCORES = 8
B, C, L = 32, 64, 8192
BPC = B // NCORES            # batches per core
M_TOT = B * L
T_NS = 5
F32 = mybir.dt.float32
F16 = mybir.dt.float16
XTILE_W = 2048
TILE_PLAN = [2048, 2048, 2048, 1536, 512]   # per pair; sums to L

_CACHE = {}


def _build_bass(ncores=NCORES):
    nc = bacc.Bacc("TRN2", target_bir_lowering=False, debug=False, num_devices=ncores)

    X = nc.dram_tensor("X", [BPC, C, L], F32, kind="ExternalInput")
    Y = nc.dram_tensor("Y", [BPC, C, L], F32, kind="ExternalOutput")
    # packed constants: one f32 and one f16 tensor (2 DMAs)
    CF32 = nc.dram_tensor("CF32", [128, 320], F32, kind="ExternalInput")
    CF16 = nc.dram_tensor("CF16", [128, 256], F16, kind="ExternalInput")

    Xv = X.ap().rearrange("(p i) c l -> p (i c) l", i=2)
    Yv = Y.ap().rearrange("(p i) c l -> p (i c) l", i=2)
    tile_geom = []
    for pair in range(2):
        off = 0
        for w in TILE_PLAN:
            tile_geom.append((pair, off, w))
            off += w
    n_chunks = 2 * L // 128
    ntiles = len(tile_geom)

    with tile.TileContext(nc) as tc:
        with (
            tc.tile_pool(name="consts", bufs=1) as consts,
            tc.tile_pool(name="xpool", bufs=4) as xpool,
            tc.tile_pool(name="xTpool", bufs=4) as xTpool,
            tc.tile_pool(name="xbpool", bufs=1) as xbpool,
            tc.tile_pool(name="ypool", bufs=6) as ypool,
            tc.tile_pool(name="small", bufs=2) as small,
            tc.tile_pool(name="psumS", bufs=1, space="PSUM") as psumS,
            tc.tile_pool(name="psumSm", bufs=3, space="PSUM") as psumSm,
            tc.tile_pool(name="psumA", bufs=4, space="PSUM") as psumA,
            tc.tile_pool(name="dramp", bufs=1, space="DRAM") as dramp,
        ):
            # ---- constants (packed: 2 DMAs, on sync ahead of the loads) ----
            cf16 = consts.tile([128, 256], F16)
            nc.sync.dma_start(cf16, CF16.ap())
            cf32 = consts.tile([128, 320], F32)
            nc.sync.dma_start(cf32, CF32.ap())
            esel = cf32[:, 0:64]            # (128,64) rows 64:128 = I64
            ident64 = cf32[0:64, 64:128]    # (64,64) I
            f2h = cf32[0:64, 128:256]       # (64,128) [I|I]
            ones64 = cf32[0:64, 256:320]    # (64,64) ones
            i15h = cf16[0:64, 0:64]         # (64,64) 1.5 I fp16
            ih16 = cf16[0:64, 64:128]       # (64,64) I fp16
            identb = cf16[:, 128:256]       # (128,128) I fp16
            W2 = consts.tile([128, 128], F16)
            nc.gpsimd.memset(W2, 0.0)

            # ---- phase 1: load + cast(+row sums) + PE transpose + Gram ----
            S_ps = psumS.tile([128, 128], F32)
            srow = consts.tile([128, 2 * ntiles], F32)

            xb_tiles = []
            gi = 0
            cpi = 0
            for t, (pair, off, w) in enumerate(tile_geom):
                nch = w // 128
                xt = xpool.tile([128, w], F32, tag="xt", name=f"xt{t}",
                                padded_shape=[128, XTILE_W])
                nc.sync.dma_start(xt, Xv[pair, :, off:off + w])
                # fp16 shadow; cast split 25/75 DVE/ACT, row sums fused
                xb = xbpool.tile([128, w], F16, tag=f"xb{t}", name=f"xb{t}", bufs=1)
                h = (nch // 4) * 128
                nc.vector.tensor_scalar(
                    xb[:, 0:h], xt[:, 0:h], 1.0, None, mybir.AluOpType.mult,
                    mybir.AluOpType.add, accum_out=srow[:, 2 * t:2 * t + 1],
                )
                nc.scalar.activation(
                    xb[:, h:w], xt[:, h:w],
                    func=mybir.ActivationFunctionType.Identity,
                    accum_out=srow[:, 2 * t + 1:2 * t + 2],
                )
                xb_tiles.append(xb)
                xTt = xTpool.tile([128, nch, 128], F16, tag="xT", name=f"xT{t}",
                                  padded_shape=[128, 16, 128])
                # PE transposes, groups of 8 chunks -> one PSUM bank -> 1 copy
                for g0 in range(0, nch, 8):
                    gn = min(8, nch - g0)
                    tp = psumA.tile([128, gn * 128], F16, tag="ap",
                                    name=f"tp{t}_{g0}", padded_shape=[128, 1024])
                    for k in range(gn):
                        nc.tensor.transpose(
                            tp[:, k * 128:(k + 1) * 128],
                            xb[:, (g0 + k) * 128:(g0 + k + 1) * 128], identb)
                    cp = tp.rearrange("p (a b) -> p a b", a=gn)
                    nc.vector.tensor_copy(xTt[:, g0:g0 + gn, :], cp)
                    cpi += 1
                for k in range(nch):
                    nc.tensor.matmul(
                        S_ps,
                        xTt[:, k, :],
                        xTt[:, k, :],
                        start=(gi == 0),
                        stop=(gi == n_chunks - 1),
                        skip_group_check=True,
                    )
                    gi += 1

            # ---- local fold to (64,65) + AllGather + on-chip reduce ----
            S_sb = small.tile([128, 129], F32, tag="ssb")
            nc.vector.tensor_copy(S_sb[:, 0:128], S_ps)
            nc.vector.tensor_reduce(
                S_sb[:, 128:129], srow, axis=mybir.AxisListType.X,
                op=mybir.AluOpType.add,
            )
            ef_ps = psumSm.tile([64, 129], F32, tag="sm", name="ef_ps")
            nc.tensor.matmul(ef_ps, esel, S_sb, start=True, stop=True)
            pack = small.tile([64, 65], F32, tag="pack")
            nc.vector.tensor_add(pack[:, 0:64], S_sb[0:64, 0:64], ef_ps[:, 64:128])
            nc.vector.tensor_add(pack[:, 64:65], S_sb[0:64, 128:129],
                                 ef_ps[:, 128:129])
            agin = dramp.tile([64, 65], F32, tag="agin")
            agout = dramp.tile([64 * ncores, 65], F32, tag="agout")
            nc.sync.dma_start(agin, pack)
            nc.gpsimd.collective_compute(
                "AllGather",
                mybir.AluOpType.bypass,
                replica_groups=[list(range(ncores))],
                ins=[agin.opt()],
                outs=[agout.opt()],
            )
            gath = small.tile([64, ncores, 65], F32, tag="gath")
            nc.sync.dma_start(
                gath, agout.rearrange("(k c) n -> c k n", k=ncores)
            )
            tot = small.tile([64, 65], F32, tag="tot")
            gv_sum = bass.AP(tensor=gath.tensor, offset=gath.offset + 64,
                             ap=[gath.ap[0], [1, 1], [65, ncores]])
            nc.vector.tensor_reduce(
                tot[:, 64:65], gv_sum, axis=mybir.AxisListType.X,
                op=mybir.AluOpType.add
            )
            gv_mat = bass.AP(tensor=gath.tensor, offset=gath.offset,
                             ap=[gath.ap[0], [1, 64], [65, ncores]])
            nc.vector.tensor_reduce(
                tot[:, 0:64], gv_mat, axis=mybir.AxisListType.X,
                op=mybir.AluOpType.add
            )

            # ---- epilogue: mu, Sigma, trace, Newton-Schulz (replicated) ----
            mu = small.tile([64, 1], F32, tag="mu")
            nc.vector.tensor_scalar_mul(mu, tot[:, 64:65], 1.0 / M_TOT)
            mur_ps = psumSm.tile([1, 64], F32, tag="sm", name="mur_ps")
            nc.tensor.matmul(mur_ps, mu, ident64, start=True, stop=True)
            mu2b_ps = psumSm.tile([128, 1], F32, tag="sm", name="mu2b_ps")
            nc.tensor.matmul(mu2b_ps, f2h, mu, start=True, stop=True)
            mu_row = small.tile([1, 64], F32, tag="murow")
            nc.vector.tensor_copy(mu_row, mur_ps)
            mu_row_negM = small.tile([1, 64], F32, tag="murown")
            nc.vector.tensor_scalar_mul(mu_row_negM, mur_ps, -float(M_TOT))
            mu2h = consts.tile([128, 1], F16)
            nc.vector.tensor_copy(mu2h, mu2b_ps)
            outer_ps = psumSm.tile([64, 64], F32, tag="sm", name="outer_ps")
            nc.tensor.matmul(outer_ps, mu_row_negM, mu_row, start=True, stop=True)
            Sigma = small.tile([64, 64], F32, tag="sigma")
            diagm = small.tile([64, 64], F32, tag="diagm")
            dred = small.tile([64, 1], F32, tag="dred")
            nc.vector.tensor_add(diagm, tot[:, 0:64], outer_ps)
            nc.vector.tensor_scalar_mul(Sigma, diagm, 1.0 / M_TOT)
            nc.vector.tensor_mul(diagm, Sigma, ident64)
            nc.vector.tensor_reduce(
                dred, diagm, axis=mybir.AxisListType.X, op=mybir.AluOpType.add
            )
            tr64_ps = psumSm.tile([64, 1], F32, tag="sm", name="tr64_ps")
            nc.tensor.matmul(tr64_ps, ones64, dred, start=True, stop=True)
            trm2 = small.tile([64, 1], F32, tag="trm2")
            nc.vector.tensor_scalar_mul(trm2, tr64_ps, -2.0)
            rtr_nh = small.tile([64, 1], F32, tag="rtrnh")   # = -0.5/tr
            nc.vector.reciprocal(rtr_nh, trm2)
            srtr = small.tile([64, 1], F32, tag="srtr")      # = sqrt(1/tr)
            nc.scalar.activation(srtr, rtr_nh,
                                 func=mybir.ActivationFunctionType.Sqrt,
                                 scale=-2.0)
            Sh = small.tile([64, 64], F16, tag="sh")         # = -0.5 Sigma_N
            nc.vector.tensor_scalar_mul(Sh, Sigma, rtr_nh)
            P = small.tile([64, 64], F16, tag="P", name="P1")
            nc.vector.tensor_add(P, i15h, Sh)                # NS iteration 1

            # ---- Newton-Schulz iterations 2..5 (fp16, paired PSUM mms) ----
            for it in range(T_NS - 1):
                last = it == T_NS - 2
                psAB = psumSm.tile([64, 128], F32, tag="sm", name=f"psAB{it}")
                nc.tensor.matmul(psAB[:, 0:64], P, P, start=True, stop=True,
                                 skip_group_check=True)
                nc.tensor.matmul(psAB[:, 64:128], P, Sh, start=True, stop=True,
                                 skip_group_check=True)
                PQ = small.tile([64, 128], F16, tag="PQ", name=f"PQ{it}")
                nc.vector.tensor_copy(PQ, psAB)
                psC = psumSm.tile([64, 64], F32, tag="sm", name=f"psC{it}")
                nc.tensor.matmul(psC, PQ[:, 0:64], PQ[:, 64:128],
                                 start=True, stop=False, skip_group_check=True)
                nc.tensor.matmul(psC, i15h, P, start=False, stop=True,
                                 skip_group_check=True)
                if last:
                    wmh = small.tile([64, 64], F16, tag="wmh")
                    nc.vector.tensor_scalar_mul(wmh, psC, srtr)
                else:
                    P = small.tile([64, 64], F16, tag="P", name=f"P{it + 2}")
                    nc.vector.tensor_copy(P, psC)

            # W2 = blockdiag(wm, wm): two identity matmuls (PE crosses
            # partitions; DVE cannot)
            W2ps = psumSm.tile([128, 64], F32, tag="sm", name="W2ps")
            nc.tensor.matmul(W2ps[0:64, :], wmh, ih16, start=True, stop=True,
                             skip_group_check=True)
            nc.tensor.matmul(W2ps[64:128, :], wmh, ih16, start=True, stop=True,
                             skip_group_check=True)
            nc.vector.tensor_copy(W2[0:64, 0:64], W2ps[0:64, :])
            nc.vector.tensor_copy(W2[64:128, 64:128], W2ps[64:128, :])
            # bias_col = -(W2 @ mu2): folded into the apply-phase copies
            bias_ps = psumSm.tile([128, 1], F32, tag="sm", name="bias_ps")
            nc.tensor.matmul(bias_ps, W2, mu2h, start=True, stop=True)
            bias_col = consts.tile([128, 1], F32)
            nc.vector.tensor_scalar_mul(bias_col, bias_ps, -1.0)

            # ---- phase 3: apply y = W2 @ xb - W2 @ mu per (128,512) chunk ----
            ci = 0
            for t, (pair, off, w) in enumerate(tile_geom):
                for cidx in range(w // 512):
                    ap_ps = psumA.tile([128, 512], F32, tag="ap",
                                       name=f"ap{t}_{cidx}",
                                       padded_shape=[128, 512])
                    nc.tensor.matmul(
                        ap_ps,
                        W2,
                        xb_tiles[t][:, cidx * 512:(cidx + 1) * 512],
                        start=True,
                        stop=True,
                    )
                    yt = ypool.tile([128, 512], F32, tag="yt",
                                    name=f"yt{t}_{cidx}")
                    if ci % 2 == 0:
                        nc.vector.tensor_scalar_add(yt, ap_ps, bias_col)
                    else:
                        nc.scalar.activation(
                            yt, ap_ps,
                            func=mybir.ActivationFunctionType.Identity,
                            bias=bias_col, scale=1.0,
                        )
                    co = off + cidx * 512
                    nc.sync.dma_start(Yv[pair, :, co:co + 512], yt)
                    ci += 1

    nc.finalize()
    return nc


def _host_consts():
    i64 = np.eye(64, dtype=np.float32)
    cf32 = np.zeros((128, 320), dtype=np.float32)
    cf32[64:128, 0:64] = i64                          # esel
    cf32[0:64, 64:128] = i64                          # ident64
    cf32[0:64, 128:192] = i64                         # f2h left
    cf32[0:64, 192:256] = i64                         # f2h right
    cf32[0:64, 256:320] = 1.0                         # ones64
    cf16 = np.zeros((128, 256), dtype=np.float16)
    cf16[0:64, 0:64] = (1.5 * i64).astype(np.float16)  # i15h
    cf16[0:64, 64:128] = i64.astype(np.float16)        # ih16
    cf16[:, 128:256] = np.eye(128, dtype=np.float16)   # identb
    return {"CF32": cf32, "CF16": cf16}


NCORES_RUN = NCORES


def _get_nc():
    key = f"nc{NCORES_RUN}"
    if key not in _CACHE:
        _CACHE[key] = _build_bass(NCORES_RUN)
    return _CACHE[key]


def run(X, **spmd_kwargs):
    """Run the SPMD kernel; returns (Y_full, BassKernelResults)."""
    X = np.ascontiguousarray(np.asarray(X), dtype=np.float32)
    assert X.shape == (B, C, L), X.shape
    nc = _get_nc()
    consts = _host_consts()
    n = NCORES_RUN
    in_maps = [
        {"X": X[c * BPC:(c + 1) * BPC], **consts} for c in range(n)
    ]
    res = run_bass_kernel_spmd(nc, in_maps, core_ids=list(range(n)), **spmd_kwargs)
    Y = np.concatenate([res.results[c]["Y"] for c in range(n)], axis=0)
    return Y, res


def kernel(X):
    Y, _ = run(X)
    return Y
